# revision 59
# baseline (speedup 1.0000x reference)
"""Causal attention (B=4, S=2048, D=1024, fp32) on 8 TRN2 NeuronCores.

Sharding: core c -> (batch b = c//2, key-parity h = c%2); all S queries, the
SH = S/2 key positions whose 128-block index has parity h (causally load
balanced). Host adds the two unnormalized partials (pout, den) per batch and
divides.

Algebra: scores = q k^T = x (Wq^T Wk) x^T. M = Wq^T Wk is precomputed on the
host (weight-only transform), so the kernel computes g = x @ M (bf16, one
projection) and uses x itself as the key matrix -- the k projection vanishes.
Mixed precision: g-proj / v-proj / A@V run in bf16 (1 col/cycle, direct
output path); the scores matmul runs as fp8e4m3 DoubleRow (2 d-block pairs
per pass, 2x throughput) where the softmax absorbs the quantization jitter --
except query chunk 0, whose rows have tiny softmax denominators (n_eff ~ a
few keys) that amplify score jitter, so chunk 0 runs in bf16 from a bf16
copy of g^T. exp() without max-subtraction: scaled scores are ~N(0, 0.17).

Engine split: PE matmuls; scalar = PSUM->SBUF casts (phases 1-2) + exp +
upper osb half; vector = mask adds, den accumulation, lower osb half; all
DMA issues on the sync ring.
"""
import numpy as np
import ml_dtypes

import concourse.bacc as bacc
import concourse.tile as tile
import concourse.mybir as mybir
from concourse import bass_utils
from concourse.tile import add_dep_helper
from contextlib import ExitStack

B, S, D = 4, 2048, 1024
SH = S // 2           # key positions per core
NU = S // 512         # 4 query chunks of 512
SCALE = 1.0 / 32.0    # 1/sqrt(D)
F32 = mybir.dt.float32
F16 = mybir.dt.float16
BF16 = mybir.dt.bfloat16
F8 = mybir.dt.float8e4
EXP = mybir.ActivationFunctionType.Exp
DRM = mybir.MatmulPerfMode.DoubleRow
NPBF16 = ml_dtypes.bfloat16
NPF8 = ml_dtypes.float8_e4m3fn

_NC = None


def _dview(ap):
    """[D, C] dram tensor -> [128, 8, C] view (partition, d-block, col)."""
    return ap.rearrange("(d p) c -> p d c", p=128)


def _build(debug=False):
    # All inputs are host-pre-swizzled into the exact SBUF tile layouts so
    # every input DMA is a linear copy with >=1 KiB contiguous runs (column
    # slices of [D, C] row-major arrays have 128-512 B runs, which the SDMA
    # engines move at a fraction of line rate).
    nc = bacc.Bacc()
    mg = nc.dram_tensor("mg", [128, 8, 8, 128], BF16, kind="ExternalInput").ap()
    mg8 = nc.dram_tensor("mg8", [128, 8, 8, 128], F8, kind="ExternalInput").ap()
    xg = nc.dram_tensor("xg", [128, 8, 512], BF16, kind="ExternalInput").ap()
    xg8 = nc.dram_tensor("xg8", [3, 128, 8, 512], F8, kind="ExternalInput").ap()
    xkb = nc.dram_tensor("xkb", [128, 8, 256], BF16, kind="ExternalInput").ap()
    xk8 = nc.dram_tensor("xk8", [128, 8, SH], F8, kind="ExternalInput").ap()
    wvb = nc.dram_tensor("wvb", [128, 8, D], BF16, kind="ExternalInput").ap()
    wv8 = nc.dram_tensor("wv8", [128, 8, D], F8, kind="ExternalInput").ap()
    dmask = nc.dram_tensor("dmask", [2, 128, 512], BF16, kind="ExternalInput").ap()
    pout = nc.dram_tensor("pout", [S, D], F16, kind="ExternalOutput").ap()
    den = nc.dram_tensor("den", [128, 2 * (S // 256)], F32, kind="ExternalOutput").ap()
    if debug:
        dbg_gt = nc.dram_tensor("dbg_gt", [128, 8, S], F8, kind="ExternalOutput").ap()
        dbg_et = nc.dram_tensor("dbg_et", [128, 8, 512], F8, kind="ExternalOutput").ap()
        dbg_vt = nc.dram_tensor("dbg_vt", [128, 8, D], BF16, kind="ExternalOutput").ap()

    def chain_to(inst, prev):
        # order-only edge: keeps ring-FIFO issue order without serializing on
        # the previous DMA's completion semaphore
        add_dep_helper(inst.ins, prev.ins, sync=False, reason="input dma ordering")
        return inst

    with tile.TileContext(nc) as tc, ExitStack() as top:
        # All long-lived tiles get fresh SBUF up front so their fill DMAs can
        # land during phase 1 (no write-after-read dep on phase-1 space).
        small = top.enter_context(tc.tile_pool(name="small", bufs=1))
        osb_pool = top.enter_context(tc.tile_pool(name="osb", bufs=2))
        gt_pool = top.enter_context(tc.tile_pool(name="gt", bufs=1))
        xkb_pool = top.enter_context(tc.tile_pool(name="xkb", bufs=1))
        wv_pool = top.enter_context(tc.tile_pool(name="wv", bufs=1))
        xk8_pool = top.enter_context(tc.tile_pool(name="xk8", bufs=1))
        v_pool = top.enter_context(tc.tile_pool(name="v", bufs=1))
        dm_pool = top.enter_context(tc.tile_pool(name="dm", bufs=1))
        et_pool = top.enter_context(tc.tile_pool(name="et", bufs=1))
        ea_pool = top.enter_context(tc.tile_pool(name="ea", bufs=1))

        gt = gt_pool.tile([128, 8, S], F8, name="gt")
        gt0b = gt_pool.tile([128, 8, 512], BF16, name="gt0b")
        xkt = xkb_pool.tile([128, 8, 256], BF16, name="xkt")
        wvt = wv_pool.tile([128, 8, D], BF16, name="wvt")
        wv8t = wv_pool.tile([128, 8, D], F8, name="wv8t")
        xk8t = xk8_pool.tile([128, 8, SH], F8, name="xk8t")
        vt = v_pool.tile([128, 8, D], BF16, name="vt")
        vt8 = v_pool.tile([128, 8, D], F8, name="vt8")
        dm = dm_pool.tile([128, 2, 512], BF16, name="dm")
        et8 = et_pool.tile([128, 8, 512], F8, name="et8")
        etb = et_pool.tile([128, 2, 512], BF16, name="etb")
        ones_f = small.tile([128, 2], F32)
        ones = small.tile([128, 2], BF16)
        den_acc = small.tile([128, 2 * (S // 256)], F32)
        junk = small.tile([128, 512], BF16)
        nc.vector.memset(ones_f, 1.0)
        nc.vector.tensor_copy(ones, ones_f)
        nc.vector.memset(junk.bitcast(F32), 0.0)

        # ---- phase 1: g^T = M-contracted x^T, for all S queries ----
        # Chunk 0 (queries 0:512) in bf16 (low-n_eff rows need precision);
        # chunks 1-3 in fp8 DoubleRow (softmax averaging over >=512 keys
        # crushes the quantization jitter). First PE group depends only on a
        # 256 KiB M e-slice plus one 512 KiB x^T half-chunk.
        # Attention-phase inputs go out on the scalar HWDGE ring (in
        # consumption order) so the sync ring serves phase 1 back-to-back.
        with ExitStack() as ph:
            mg_pool = ph.enter_context(tc.tile_pool(name="mg", bufs=1))
            xs_pool = ph.enter_context(tc.tile_pool(name="xs", bufs=2))
            xs0_pool = ph.enter_context(tc.tile_pool(name="xs0", bufs=1))
            psB = ph.enter_context(tc.tile_pool(name="psB", bufs=4, space="PSUM"))
            warm_ps = ph.enter_context(tc.tile_pool(name="warm", bufs=1, space="PSUM"))
            wp = warm_ps.tile([128, 512], F32, name="wp")
            for _ in range(20):
                # warm HAM to 8/8 during the initial DMA wait
                nc.tensor.matmul(wp[0:2, :], lhsT=junk[:, 0:2], rhs=junk,
                                 start=True, stop=True, skip_group_check=True)
            mgt = mg_pool.tile([128, 8, 8, 128], BF16, name="mgt")
            mg8t = mg_pool.tile([128, 8, 8, 128], F8, name="mg8t")
            xs0 = xs0_pool.tile([128, 8, 512], BF16, name="xs0")
            xs_dma = [None] * 4
            xs_tiles = [None] * 4
            for c in range(1, 4):
                xs_tiles[c] = xs_pool.tile([128, 8, 512], F8, name="xs8")
            # A single DMA on this part moves at only ~45 GB/s; bandwidth
            # comes from several in-flight DMAs and from the three issue
            # rings (sync / scalar HWDGE + gpsimd SWDGE) running in parallel.
            # bf16 chunk 0 computes first, so its 3 MiB is split across all
            # three rings by e-slice deadline; the fp8 chunks follow with
            # ~20 us of slack, then the attention inputs.
            # each ring moves ~45-90 GB/s serially; the three first-MM deps
            # (mgt e0, xs0 halves) go on three different rings
            sy = nc.sync.dma_start(out=mgt[:, 0:1], in_=mg[:, 0:1])
            sc = nc.scalar.dma_start(out=xs0[:, 4:8, :], in_=xg[:, 4:8, :])
            gp = nc.gpsimd.dma_start(out=xs0[:, 0:4, :], in_=xg[:, 0:4, :])
            sy = chain_to(nc.sync.dma_start(out=mgt[:, 1:3], in_=mg[:, 1:3]), sy)
            sc = chain_to(nc.scalar.dma_start(out=mgt[:, 5:8], in_=mg[:, 5:8]), sc)
            sy = chain_to(nc.sync.dma_start(out=mgt[:, 3:5], in_=mg[:, 3:5]), sy)
            # fp8 g-proj inputs (consumed from ~+28 us)
            gp = chain_to(nc.gpsimd.dma_start(
                out=xs_tiles[1][:, 0:4, :], in_=xg8[0][:, 0:4, :]), gp)
            gp = chain_to(nc.gpsimd.dma_start(
                out=xs_tiles[1][:, 4:8, :], in_=xg8[0][:, 4:8, :]), gp)
            sc = chain_to(nc.scalar.dma_start(out=mg8t[:, 0], in_=mg8[:, 0]), sc)
            sc = chain_to(nc.scalar.dma_start(out=mg8t[:, 1:4], in_=mg8[:, 1:4]), sc)
            sc = chain_to(nc.scalar.dma_start(out=mg8t[:, 4:8], in_=mg8[:, 4:8]), sc)
            sy = xs_dma[2] = chain_to(nc.sync.dma_start(out=xs_tiles[2], in_=xg8[1]), sy)
            sy = xs_dma[3] = chain_to(nc.sync.dma_start(out=xs_tiles[3], in_=xg8[2]), sy)
            # attention inputs: Wv first on gpsimd (v-proj runs fp8 blocks
            # before bf16 blocks), keys later
            gp = chain_to(nc.gpsimd.dma_start(out=wv8t, in_=wv8), gp)
            gp = chain_to(nc.gpsimd.dma_start(out=wvt[:, 0:4, :], in_=wvb[:, 0:4, :]), gp)
            gp = chain_to(nc.gpsimd.dma_start(out=wvt[:, 4:8, :], in_=wvb[:, 4:8, :]), gp)
            gp = chain_to(nc.gpsimd.dma_start(out=xkt, in_=xkb), gp)
            sc = chain_to(nc.scalar.dma_start(out=xk8t, in_=xk8), sc)
            dmd = chain_to(nc.scalar.dma_start(out=dm[:, 0, :], in_=dmask[0]), sc)
            chain_to(nc.scalar.dma_start(out=dm[:, 1, :], in_=dmask[1]), dmd)
            for c in (0, 1, 2, 3):
                for e in range(8):
                    ps = psB.tile([128, 512], F32)
                    if c == 0:
                        for d_ in range(8):
                            nc.tensor.matmul(ps, lhsT=mgt[:, e, d_, :],
                                             rhs=xs0[:, d_, :], start=d_ == 0, stop=d_ == 7)
                        # chunk 0 is consumed only via the bf16 copy (u=0 path)
                        nc.vector.tensor_copy(gt0b[:, e, :], ps)
                    else:
                        for i in range(4):
                            nc.tensor.matmul(
                                ps, lhsT=mg8t[:, e, 2 * i:2 * i + 2, :],
                                rhs=xs_tiles[c][:, 2 * i:2 * i + 2, :],
                                start=i == 0, stop=i == 3, perf_mode=DRM)
                        nc.scalar.copy(gt[:, e, c * 512:(c + 1) * 512], ps)

        # ---- v projection: local key blocks 0-1 (global keys < 512, seen by
        # precision-sensitive early queries) in bf16; blocks 2+ (only ever
        # weighted by diffuse queries >= 512) in fp8 DoubleRow ----
        with ExitStack() as ph:
            psA2 = ph.enter_context(tc.tile_pool(name="psA2", bufs=4, space="PSUM"))
            for ec in range(2):
                for s_ in (2, 3, 4, 5, 6, 7, 0, 1):
                    ps = psA2.tile([128, 512], F32)
                    if s_ < 2:
                        for d_ in range(8):
                            nc.tensor.matmul(
                                ps, lhsT=xkt[:, d_, s_ * 128:(s_ + 1) * 128],
                                rhs=wvt[:, d_, ec * 512:(ec + 1) * 512],
                                start=d_ == 0, stop=d_ == 7)
                        nc.scalar.copy(vt[:, s_, ec * 512:(ec + 1) * 512], ps)
                    else:
                        for i in range(4):
                            nc.tensor.matmul(
                                ps, lhsT=xk8t[:, 2 * i:2 * i + 2, s_ * 128:(s_ + 1) * 128],
                                rhs=wv8t[:, 2 * i:2 * i + 2, ec * 512:(ec + 1) * 512],
                                start=i == 0, stop=i == 3, perf_mode=DRM)
                    nc.vector.tensor_copy(vt8[:, s_, ec * 512:(ec + 1) * 512], ps)

        # ---- attention over 512-query chunks, largest first ----
        ps_sc = top.enter_context(tc.tile_pool(name="ps_sc", bufs=2, space="PSUM"))
        ps_out = top.enter_context(tc.tile_pool(name="ps_out", bufs=1, space="PSUM"))
        ps_den = top.enter_context(tc.tile_pool(name="ps_den", bufs=1, space="PSUM"))
        if debug:
            nc.sync.dma_start(out=dbg_gt, in_=gt)
            nc.sync.dma_start(out=dbg_vt, in_=vt)

        blk_ctr = [0]

        def av_pass(u, qs, jmax, eaccb):
            """A@V + den + drain for q128 slices `qs`, k-blocks 0..jmax.

            u >= 1 runs fp8 DoubleRow over k-block pairs (slots 0..2u+1; the
            top slot is exp(masked)=0 for the q01 pass, a harmless zero
            contribution). u == 0 runs bf16 (precision floor rows).
            """
            outp = [[ps_out.tile([128, 512], F32, tag=f"po{q & 1}{ec}", name=f"po{q & 1}{ec}")
                     for ec in range(2)] for q in qs]
            denp = {}
            for qi, q in enumerate(qs):
                # den only needs eaccb -- run it ahead of the AV matmuls so
                # the post-AV drain chain is just copies + DMA
                denp[q] = ps_den.tile([128, 2], F32, tag=f"pd{q & 1}", name=f"pd{q & 1}")
                nc.tensor.matmul(denp[q], lhsT=eaccb[:, q * 128:(q + 1) * 128],
                                 rhs=ones, start=True, stop=True)
                nc.vector.tensor_copy(den_acc[:, 4 * u + q:4 * u + q + 1], denp[q][:, 0:1])
            if u > 0:
                for m in range(u + 1):
                    for qi, q in enumerate(qs):
                        for ec in range(2):
                            nc.tensor.matmul(
                                outp[qi][ec],
                                lhsT=et8[:, 2 * m:2 * m + 2, q * 128:(q + 1) * 128],
                                rhs=vt8[:, 2 * m:2 * m + 2, ec * 512:(ec + 1) * 512],
                                start=m == 0, stop=m == u, perf_mode=DRM)
            else:
                for jj in range(jmax + 1):
                    for qi, q in enumerate(qs):
                        for ec in range(2):
                            nc.tensor.matmul(
                                outp[qi][ec], lhsT=etb[:, jj, q * 128:(q + 1) * 128],
                                rhs=vt[:, jj, ec * 512:(ec + 1) * 512],
                                start=jj == 0, stop=jj == jmax)
            for qi, q in enumerate(qs):
                row = u * 512 + q * 128
                osb = osb_pool.tile([128, D], F16, tag="osb", name="osb")
                nc.vector.tensor_copy(osb[:, 0:512], outp[qi][0])
                nc.scalar.copy(osb[:, 512:1024], outp[qi][1])
                # output is ring-bandwidth-bound: spread the halves over the
                # three issue rings round-robin
                k = blk_ctr[0]
                blk_ctr[0] += 1
                engs = (nc.sync, nc.scalar, nc.gpsimd)
                if k >= 14:
                    # final blocks: quarter-splits so the tail drains fast
                    for t in range(4):
                        engs[(k + t) % 3].dma_start(
                            out=pout[row:row + 128, t * 256:(t + 1) * 256],
                            in_=osb[:, t * 256:(t + 1) * 256])
                else:
                    engs[k % 3].dma_start(out=pout[row:row + 128, 0:512], in_=osb[:, 0:512])
                    engs[(k + 1) % 3].dma_start(out=pout[row:row + 128, 512:D], in_=osb[:, 512:D])

        for u in (3, 2, 0, 1):
            eacc = ea_pool.tile([128, 512], F32, tag=f"ea{u & 1}", name=f"ea{u & 1}")
            eaccb = ea_pool.tile([128, 512], BF16, tag=f"eb{u & 1}", name=f"eb{u & 1}")
            for jj in range(2 * u + 2):
                sp = ps_sc.tile([128, 512], F32)
                if u == 0:
                    # bf16 scores for the small-denominator query rows
                    for e_ in range(8):
                        nc.tensor.matmul(
                            sp, lhsT=xkt[:, e_, jj * 128:(jj + 1) * 128],
                            rhs=gt0b[:, e_, :], start=e_ == 0, stop=e_ == 7)
                else:
                    for i in range(4):
                        nc.tensor.matmul(
                            sp, lhsT=xk8t[:, 2 * i:2 * i + 2, jj * 128:(jj + 1) * 128],
                            rhs=gt[:, 2 * i:2 * i + 2, u * 512:(u + 1) * 512],
                            start=i == 0, stop=i == 3, perf_mode=DRM)
                if jj == 2 * u:
                    nc.vector.tensor_add(sp, sp, dm[:, 0, :])
                elif jj == 2 * u + 1:
                    nc.vector.tensor_add(sp, sp, dm[:, 1, :])
                et = etb if u == 0 else et8
                nc.scalar.activation(et[:, jj, :], sp, EXP, scale=SCALE)
                if jj == 0:
                    nc.vector.tensor_copy(eacc, et[:, 0, :])
                else:
                    nc.vector.tensor_add(eacc, eacc, et[:, jj, :])
            nc.vector.tensor_copy(eaccb, eacc)
            if debug and u == 3:
                nc.sync.dma_start(out=dbg_et, in_=et8)
            av_pass(u, (0, 1), 2 * u, eaccb)
            av_pass(u, (2, 3), 2 * u + 1, eaccb)
            nc.sync.dma_start(out=den[:, 4 * u:4 * u + 4], in_=den_acc[:, 4 * u:4 * u + 4])

    nc.compile()
    return nc


def _sw(a):
    """[D, C] -> [128, 8, C] (partition, d-block, col), contiguous."""
    return np.ascontiguousarray(a.reshape(8, 128, -1).transpose(1, 0, 2))


def _prep_inputs(x, Wq, Wk, Wv):
    M = (Wq.T @ Wk).astype(np.float32)          # [D, D] (d, e)
    # mg[p, e, d, j] = M[d*128+p, e*128+j]
    mgblk = np.ascontiguousarray(M.reshape(8, 128, 8, 128).transpose(1, 2, 0, 3))
    mgb = mgblk.astype(NPBF16)
    mg8s = mgblk.astype(NPF8)
    wvTs = _sw(np.ascontiguousarray(Wv.T))
    wv_b = wvTs.astype(NPBF16)
    wv_b8 = wvTs.astype(NPF8)
    i = np.arange(128)[:, None]
    j = np.arange(512)[None, :]
    in_maps = []
    for c in range(8):
        b, h = c // 2, c % 2
        xb = x[b]                                   # [S, D]
        xTs = _sw(np.ascontiguousarray(xb.T))       # [128, 8, S]
        xk = xb.reshape(S // 128, 128, D)[h::2].reshape(SH, D)
        xkTs = _sw(np.ascontiguousarray(xk.T))      # [128, 8, S/2]
        xkb_s = np.ascontiguousarray(xkTs[:, :, 0:256])
        # xg8[c, p, d, j] = xT[., 512 + c*512 + j]
        xg8s = np.ascontiguousarray(
            xTs[:, :, 512:].reshape(128, 8, 3, 512).transpose(2, 0, 1, 3))
        dm_a = np.where(j >= i + 128 * h, np.float32(0.0), np.float32(-1e30))
        dm_b = np.where(j >= 256 + i + 128 * h, np.float32(0.0), np.float32(-1e30))
        dmask = np.stack([dm_a, dm_b]).astype(np.float32)
        in_maps.append({
            "mg": mgb, "mg8": mg8s,
            "xg": np.ascontiguousarray(xTs[:, :, 0:512]).astype(NPBF16),
            "xg8": xg8s.astype(NPF8),
            "xkb": xkb_s.astype(NPBF16), "xk8": xkTs.astype(NPF8),
            "wvb": wv_b, "wv8": wv_b8,
            "dmask": np.ascontiguousarray(dmask).astype(NPBF16),
        })
    return in_maps


def _run(inputs, trace=False, **kw):
    global _NC
    if _NC is None:
        _NC = _build()
    x = np.asarray(inputs["x"], dtype=np.float32)
    Wq = np.asarray(inputs["Wq"], dtype=np.float32)
    Wk = np.asarray(inputs["Wk"], dtype=np.float32)
    Wv = np.asarray(inputs["Wv"], dtype=np.float32)
    in_maps = _prep_inputs(x, Wq, Wk, Wv)
    res = bass_utils.run_bass_kernel_spmd(
        _NC, in_maps, core_ids=list(range(8)), trace=trace, **kw)
    out = np.empty((B, S, D), dtype=np.float32)
    for b in range(B):
        po = (res.results[2 * b]["pout"].astype(np.float32)
              + res.results[2 * b + 1]["pout"].astype(np.float32))
        dn = res.results[2 * b]["den"] + res.results[2 * b + 1]["den"]
        out[b] = po / dn.T.reshape(S, 1)
    return out, res


def kernel(**inputs):
    out, _ = _run(inputs, trace=False)
    return out


# revision 61
# speedup vs baseline: 1.0300x; 1.0300x over previous
"""Causal attention (B=4, S=2048, D=1024, fp32) on 8 TRN2 NeuronCores.

Sharding: core c -> (batch b = c//2, key-parity h = c%2); all S queries, the
SH = S/2 key positions whose 128-block index has parity h (causally load
balanced). Host adds the two unnormalized partials (pout, den) per batch and
divides.

Algebra: scores = q k^T = x (Wq^T Wk) x^T. M = Wq^T Wk is precomputed on the
host (weight-only transform), so the kernel computes g = x @ M (bf16, one
projection) and uses x itself as the key matrix -- the k projection vanishes.
Mixed precision: g-proj / v-proj / A@V run in bf16 (1 col/cycle, direct
output path); the scores matmul runs as fp8e4m3 DoubleRow (2 d-block pairs
per pass, 2x throughput) where the softmax absorbs the quantization jitter --
except query chunk 0, whose rows have tiny softmax denominators (n_eff ~ a
few keys) that amplify score jitter, so chunk 0 runs in bf16 from a bf16
copy of g^T. exp() without max-subtraction: scaled scores are ~N(0, 0.17).

Engine split: PE matmuls; scalar = PSUM->SBUF casts (phases 1-2) + exp +
upper osb half; vector = mask adds, den accumulation, lower osb half; all
DMA issues on the sync ring.
"""
import numpy as np
import ml_dtypes

import concourse.bacc as bacc
import concourse.tile as tile
import concourse.mybir as mybir
from concourse import bass_utils
from concourse.tile import add_dep_helper
from contextlib import ExitStack

B, S, D = 4, 2048, 1024
SH = S // 2           # key positions per core
NU = S // 512         # 4 query chunks of 512
SCALE = 1.0 / 32.0    # 1/sqrt(D)
F32 = mybir.dt.float32
F16 = mybir.dt.float16
BF16 = mybir.dt.bfloat16
F8 = mybir.dt.float8e4
EXP = mybir.ActivationFunctionType.Exp
DRM = mybir.MatmulPerfMode.DoubleRow
NPBF16 = ml_dtypes.bfloat16
NPF8 = ml_dtypes.float8_e4m3fn

_NC = None


def _dview(ap):
    """[D, C] dram tensor -> [128, 8, C] view (partition, d-block, col)."""
    return ap.rearrange("(d p) c -> p d c", p=128)


def _build(debug=False):
    # All inputs are host-pre-swizzled into the exact SBUF tile layouts so
    # every input DMA is a linear copy with >=1 KiB contiguous runs (column
    # slices of [D, C] row-major arrays have 128-512 B runs, which the SDMA
    # engines move at a fraction of line rate).
    nc = bacc.Bacc()
    mg = nc.dram_tensor("mg", [128, 8, 8, 128], BF16, kind="ExternalInput").ap()
    mg8 = nc.dram_tensor("mg8", [128, 8, 8, 128], F8, kind="ExternalInput").ap()
    xg = nc.dram_tensor("xg", [128, 8, 512], BF16, kind="ExternalInput").ap()
    xg8 = nc.dram_tensor("xg8", [3, 128, 8, 512], F8, kind="ExternalInput").ap()
    xkb = nc.dram_tensor("xkb", [128, 8, 256], BF16, kind="ExternalInput").ap()
    xk8 = nc.dram_tensor("xk8", [128, 8, SH], F8, kind="ExternalInput").ap()
    wvb = nc.dram_tensor("wvb", [128, 8, D], BF16, kind="ExternalInput").ap()
    wv8 = nc.dram_tensor("wv8", [128, 8, D], F8, kind="ExternalInput").ap()
    dmask = nc.dram_tensor("dmask", [2, 128, 512], BF16, kind="ExternalInput").ap()
    pout = nc.dram_tensor("pout", [S, D], F16, kind="ExternalOutput").ap()
    den = nc.dram_tensor("den", [128, 2 * (S // 256)], F32, kind="ExternalOutput").ap()
    if debug:
        dbg_gt = nc.dram_tensor("dbg_gt", [128, 8, S], F8, kind="ExternalOutput").ap()
        dbg_et = nc.dram_tensor("dbg_et", [128, 8, 512], F8, kind="ExternalOutput").ap()
        dbg_vt = nc.dram_tensor("dbg_vt", [128, 8, D], BF16, kind="ExternalOutput").ap()

    def chain_to(inst, prev):
        # order-only edge: keeps ring-FIFO issue order without serializing on
        # the previous DMA's completion semaphore
        add_dep_helper(inst.ins, prev.ins, sync=False, reason="input dma ordering")
        return inst

    with tile.TileContext(nc) as tc, ExitStack() as top:
        # All long-lived tiles get fresh SBUF up front so their fill DMAs can
        # land during phase 1 (no write-after-read dep on phase-1 space).
        small = top.enter_context(tc.tile_pool(name="small", bufs=1))
        osb_pool = top.enter_context(tc.tile_pool(name="osb", bufs=2))
        gt_pool = top.enter_context(tc.tile_pool(name="gt", bufs=1))
        xkb_pool = top.enter_context(tc.tile_pool(name="xkb", bufs=1))
        wv_pool = top.enter_context(tc.tile_pool(name="wv", bufs=1))
        xk8_pool = top.enter_context(tc.tile_pool(name="xk8", bufs=1))
        v_pool = top.enter_context(tc.tile_pool(name="v", bufs=1))
        dm_pool = top.enter_context(tc.tile_pool(name="dm", bufs=1))
        et_pool = top.enter_context(tc.tile_pool(name="et", bufs=1))
        ea_pool = top.enter_context(tc.tile_pool(name="ea", bufs=1))

        gt = gt_pool.tile([128, 8, S], F8, name="gt")
        gt0b = gt_pool.tile([128, 8, 512], BF16, name="gt0b")
        xkt = xkb_pool.tile([128, 8, 256], BF16, name="xkt")
        wvt = wv_pool.tile([128, 8, D], BF16, name="wvt")
        wv8t = wv_pool.tile([128, 8, D], F8, name="wv8t")
        xk8t = xk8_pool.tile([128, 8, SH], F8, name="xk8t")
        vt = v_pool.tile([128, 8, D], BF16, name="vt")
        vt8 = v_pool.tile([128, 8, D], F8, name="vt8")
        dm = dm_pool.tile([128, 2, 512], BF16, name="dm")
        et8 = et_pool.tile([128, 8, 512], F8, name="et8")
        etb = et_pool.tile([128, 2, 512], BF16, name="etb")
        ones_f = small.tile([128, 2], F32)
        ones = small.tile([128, 2], BF16)
        den_acc = small.tile([128, 2 * (S // 256)], F32)
        junk = small.tile([128, 512], BF16)
        nc.vector.memset(ones_f, 1.0)
        nc.vector.tensor_copy(ones, ones_f)
        nc.vector.memset(junk.bitcast(F32), 0.0)

        # ---- phase 1: g^T = M-contracted x^T, for all S queries ----
        # Chunk 0 (queries 0:512) in bf16 (low-n_eff rows need precision);
        # chunks 1-3 in fp8 DoubleRow (softmax averaging over >=512 keys
        # crushes the quantization jitter). First PE group depends only on a
        # 256 KiB M e-slice plus one 512 KiB x^T half-chunk.
        # Attention-phase inputs go out on the scalar HWDGE ring (in
        # consumption order) so the sync ring serves phase 1 back-to-back.
        with ExitStack() as ph:
            mg_pool = ph.enter_context(tc.tile_pool(name="mg", bufs=1))
            xs_pool = ph.enter_context(tc.tile_pool(name="xs", bufs=2))
            xs0_pool = ph.enter_context(tc.tile_pool(name="xs0", bufs=1))
            psB = ph.enter_context(tc.tile_pool(name="psB", bufs=4, space="PSUM"))
            warm_ps = ph.enter_context(tc.tile_pool(name="warm", bufs=1, space="PSUM"))
            wp = warm_ps.tile([128, 512], F32, name="wp")
            for _ in range(36):
                # warm HAM to 8/8 during the initial DMA wait (~16 us:
                # ~8 cold matmuls @427ns flip HAM, the rest stream @213ns)
                nc.tensor.matmul(wp[0:2, :], lhsT=junk[:, 0:2], rhs=junk,
                                 start=True, stop=True, skip_group_check=True)
            mgt = mg_pool.tile([128, 8, 8, 128], BF16, name="mgt")
            mg8t = mg_pool.tile([128, 8, 8, 128], F8, name="mg8t")
            xs0 = xs0_pool.tile([128, 8, 512], BF16, name="xs0")
            xs_dma = [None] * 4
            xs_tiles = [None] * 4
            for c in range(1, 4):
                xs_tiles[c] = xs_pool.tile([128, 8, 512], F8, name="xs8")
            # A single DMA on this part moves at only ~45 GB/s; bandwidth
            # comes from several in-flight DMAs and from the three issue
            # rings (sync / scalar HWDGE + gpsimd SWDGE) running in parallel.
            # bf16 chunk 0 computes first, so its 3 MiB is split across all
            # three rings by e-slice deadline; the fp8 chunks follow with
            # ~20 us of slack, then the attention inputs.
            # each ring moves ~45-90 GB/s serially; the three first-MM deps
            # (mgt e0, xs0 halves) go on three different rings
            sy = nc.sync.dma_start(out=mgt[:, 0:1], in_=mg[:, 0:1])
            sc = nc.scalar.dma_start(out=xs0[:, 4:8, :], in_=xg[:, 4:8, :])
            gp = nc.gpsimd.dma_start(out=xs0[:, 0:4, :], in_=xg[:, 0:4, :])
            sy = chain_to(nc.sync.dma_start(out=mgt[:, 1:3], in_=mg[:, 1:3]), sy)
            sc = chain_to(nc.scalar.dma_start(out=mgt[:, 5:8], in_=mg[:, 5:8]), sc)
            sy = chain_to(nc.sync.dma_start(out=mgt[:, 3:5], in_=mg[:, 3:5]), sy)
            # fp8 g-proj inputs (consumed from ~+28 us)
            gp = chain_to(nc.gpsimd.dma_start(
                out=xs_tiles[1][:, 0:4, :], in_=xg8[0][:, 0:4, :]), gp)
            gp = chain_to(nc.gpsimd.dma_start(
                out=xs_tiles[1][:, 4:8, :], in_=xg8[0][:, 4:8, :]), gp)
            sc = chain_to(nc.scalar.dma_start(out=mg8t[:, 0], in_=mg8[:, 0]), sc)
            sc = chain_to(nc.scalar.dma_start(out=mg8t[:, 1:4], in_=mg8[:, 1:4]), sc)
            sc = chain_to(nc.scalar.dma_start(out=mg8t[:, 4:8], in_=mg8[:, 4:8]), sc)
            sy = xs_dma[2] = chain_to(nc.sync.dma_start(out=xs_tiles[2], in_=xg8[1]), sy)
            sy = xs_dma[3] = chain_to(nc.sync.dma_start(out=xs_tiles[3], in_=xg8[2]), sy)
            # attention inputs: Wv first on gpsimd (v-proj runs fp8 blocks
            # before bf16 blocks), keys later
            gp = chain_to(nc.gpsimd.dma_start(out=wv8t, in_=wv8), gp)
            gp = chain_to(nc.gpsimd.dma_start(out=wvt[:, 0:4, :], in_=wvb[:, 0:4, :]), gp)
            gp = chain_to(nc.gpsimd.dma_start(out=wvt[:, 4:8, :], in_=wvb[:, 4:8, :]), gp)
            gp = chain_to(nc.gpsimd.dma_start(out=xkt, in_=xkb), gp)
            sc = chain_to(nc.scalar.dma_start(out=xk8t, in_=xk8), sc)
            dmd = chain_to(nc.scalar.dma_start(out=dm[:, 0, :], in_=dmask[0]), sc)
            chain_to(nc.scalar.dma_start(out=dm[:, 1, :], in_=dmask[1]), dmd)
            for c in (0, 1, 2, 3):
                for e in range(8):
                    ps = psB.tile([128, 512], F32)
                    if c == 0:
                        for d_ in range(8):
                            nc.tensor.matmul(ps, lhsT=mgt[:, e, d_, :],
                                             rhs=xs0[:, d_, :], start=d_ == 0, stop=d_ == 7)
                        # chunk 0 is consumed only via the bf16 copy (u=0 path)
                        nc.vector.tensor_copy(gt0b[:, e, :], ps)
                    else:
                        for i in range(4):
                            nc.tensor.matmul(
                                ps, lhsT=mg8t[:, e, 2 * i:2 * i + 2, :],
                                rhs=xs_tiles[c][:, 2 * i:2 * i + 2, :],
                                start=i == 0, stop=i == 3, perf_mode=DRM)
                        nc.scalar.copy(gt[:, e, c * 512:(c + 1) * 512], ps)

        # ---- v projection: local key blocks 0-1 (global keys < 512, seen by
        # precision-sensitive early queries) in bf16; blocks 2+ (only ever
        # weighted by diffuse queries >= 512) in fp8 DoubleRow ----
        with ExitStack() as ph:
            psA2 = ph.enter_context(tc.tile_pool(name="psA2", bufs=4, space="PSUM"))
            for ec in range(2):
                for s_ in (2, 3, 4, 5, 6, 7, 0, 1):
                    ps = psA2.tile([128, 512], F32)
                    if s_ < 2:
                        for d_ in range(8):
                            nc.tensor.matmul(
                                ps, lhsT=xkt[:, d_, s_ * 128:(s_ + 1) * 128],
                                rhs=wvt[:, d_, ec * 512:(ec + 1) * 512],
                                start=d_ == 0, stop=d_ == 7)
                        nc.scalar.copy(vt[:, s_, ec * 512:(ec + 1) * 512], ps)
                    else:
                        for i in range(4):
                            nc.tensor.matmul(
                                ps, lhsT=xk8t[:, 2 * i:2 * i + 2, s_ * 128:(s_ + 1) * 128],
                                rhs=wv8t[:, 2 * i:2 * i + 2, ec * 512:(ec + 1) * 512],
                                start=i == 0, stop=i == 3, perf_mode=DRM)
                    nc.vector.tensor_copy(vt8[:, s_, ec * 512:(ec + 1) * 512], ps)

        # ---- attention over 512-query chunks, largest first ----
        ps_sc = top.enter_context(tc.tile_pool(name="ps_sc", bufs=2, space="PSUM"))
        ps_out = top.enter_context(tc.tile_pool(name="ps_out", bufs=1, space="PSUM"))
        ps_den = top.enter_context(tc.tile_pool(name="ps_den", bufs=1, space="PSUM"))
        if debug:
            nc.sync.dma_start(out=dbg_gt, in_=gt)
            nc.sync.dma_start(out=dbg_vt, in_=vt)

        blk_ctr = [0]

        def av_pass(u, qs, jmax, eaccb):
            """A@V + den + drain for q128 slices `qs`, k-blocks 0..jmax.

            u >= 1 runs fp8 DoubleRow over k-block pairs (slots 0..2u+1; the
            top slot is exp(masked)=0 for the q01 pass, a harmless zero
            contribution). u == 0 runs bf16 (precision floor rows).
            """
            outp = [[ps_out.tile([128, 512], F32, tag=f"po{q & 1}{ec}", name=f"po{q & 1}{ec}")
                     for ec in range(2)] for q in qs]
            denp = {}
            for qi, q in enumerate(qs):
                # den only needs eaccb -- run it ahead of the AV matmuls so
                # the post-AV drain chain is just copies + DMA
                denp[q] = ps_den.tile([128, 2], F32, tag=f"pd{q & 1}", name=f"pd{q & 1}")
                nc.tensor.matmul(denp[q], lhsT=eaccb[:, q * 128:(q + 1) * 128],
                                 rhs=ones, start=True, stop=True)
                nc.vector.tensor_copy(den_acc[:, 4 * u + q:4 * u + q + 1], denp[q][:, 0:1])
            if u > 0:
                for m in range(u + 1):
                    for qi, q in enumerate(qs):
                        for ec in range(2):
                            nc.tensor.matmul(
                                outp[qi][ec],
                                lhsT=et8[:, 2 * m:2 * m + 2, q * 128:(q + 1) * 128],
                                rhs=vt8[:, 2 * m:2 * m + 2, ec * 512:(ec + 1) * 512],
                                start=m == 0, stop=m == u, perf_mode=DRM)
            else:
                for jj in range(jmax + 1):
                    for qi, q in enumerate(qs):
                        for ec in range(2):
                            nc.tensor.matmul(
                                outp[qi][ec], lhsT=etb[:, jj, q * 128:(q + 1) * 128],
                                rhs=vt[:, jj, ec * 512:(ec + 1) * 512],
                                start=jj == 0, stop=jj == jmax)
            for qi, q in enumerate(qs):
                row = u * 512 + q * 128
                osb = osb_pool.tile([128, D], F16, tag="osb", name="osb")
                nc.vector.tensor_copy(osb[:, 0:512], outp[qi][0])
                nc.scalar.copy(osb[:, 512:1024], outp[qi][1])
                # output is ring-bandwidth-bound: spread the halves over the
                # three issue rings round-robin
                k = blk_ctr[0]
                blk_ctr[0] += 1
                engs = (nc.sync, nc.scalar, nc.gpsimd)
                engs[k % 3].dma_start(out=pout[row:row + 128, 0:512], in_=osb[:, 0:512])
                engs[(k + 1) % 3].dma_start(out=pout[row:row + 128, 512:D], in_=osb[:, 512:D])

        for u in (3, 2, 0, 1):
            eacc = ea_pool.tile([128, 512], F32, tag=f"ea{u & 1}", name=f"ea{u & 1}")
            eaccb = ea_pool.tile([128, 512], BF16, tag=f"eb{u & 1}", name=f"eb{u & 1}")
            for jj in range(2 * u + 2):
                sp = ps_sc.tile([128, 512], F32)
                if u == 0:
                    # bf16 scores for the small-denominator query rows
                    for e_ in range(8):
                        nc.tensor.matmul(
                            sp, lhsT=xkt[:, e_, jj * 128:(jj + 1) * 128],
                            rhs=gt0b[:, e_, :], start=e_ == 0, stop=e_ == 7)
                else:
                    for i in range(4):
                        nc.tensor.matmul(
                            sp, lhsT=xk8t[:, 2 * i:2 * i + 2, jj * 128:(jj + 1) * 128],
                            rhs=gt[:, 2 * i:2 * i + 2, u * 512:(u + 1) * 512],
                            start=i == 0, stop=i == 3, perf_mode=DRM)
                if jj == 2 * u:
                    nc.vector.tensor_add(sp, sp, dm[:, 0, :])
                elif jj == 2 * u + 1:
                    nc.vector.tensor_add(sp, sp, dm[:, 1, :])
                et = etb if u == 0 else et8
                nc.scalar.activation(et[:, jj, :], sp, EXP, scale=SCALE)
                if jj == 0:
                    nc.vector.tensor_copy(eacc, et[:, 0, :])
                else:
                    nc.vector.tensor_add(eacc, eacc, et[:, jj, :])
            nc.vector.tensor_copy(eaccb, eacc)
            if debug and u == 3:
                nc.sync.dma_start(out=dbg_et, in_=et8)
            av_pass(u, (0, 1), 2 * u, eaccb)
            av_pass(u, (2, 3), 2 * u + 1, eaccb)
            nc.sync.dma_start(out=den[:, 4 * u:4 * u + 4], in_=den_acc[:, 4 * u:4 * u + 4])

    nc.compile()
    return nc


def _sw(a):
    """[D, C] -> [128, 8, C] (partition, d-block, col), contiguous."""
    return np.ascontiguousarray(a.reshape(8, 128, -1).transpose(1, 0, 2))


def _prep_inputs(x, Wq, Wk, Wv):
    M = (Wq.T @ Wk).astype(np.float32)          # [D, D] (d, e)
    # mg[p, e, d, j] = M[d*128+p, e*128+j]
    mgblk = np.ascontiguousarray(M.reshape(8, 128, 8, 128).transpose(1, 2, 0, 3))
    mgb = mgblk.astype(NPBF16)
    mg8s = mgblk.astype(NPF8)
    wvTs = _sw(np.ascontiguousarray(Wv.T))
    wv_b = wvTs.astype(NPBF16)
    wv_b8 = wvTs.astype(NPF8)
    i = np.arange(128)[:, None]
    j = np.arange(512)[None, :]
    in_maps = []
    for c in range(8):
        b, h = c // 2, c % 2
        xb = x[b]                                   # [S, D]
        xTs = _sw(np.ascontiguousarray(xb.T))       # [128, 8, S]
        xk = xb.reshape(S // 128, 128, D)[h::2].reshape(SH, D)
        xkTs = _sw(np.ascontiguousarray(xk.T))      # [128, 8, S/2]
        xkb_s = np.ascontiguousarray(xkTs[:, :, 0:256])
        # xg8[c, p, d, j] = xT[., 512 + c*512 + j]
        xg8s = np.ascontiguousarray(
            xTs[:, :, 512:].reshape(128, 8, 3, 512).transpose(2, 0, 1, 3))
        dm_a = np.where(j >= i + 128 * h, np.float32(0.0), np.float32(-1e30))
        dm_b = np.where(j >= 256 + i + 128 * h, np.float32(0.0), np.float32(-1e30))
        dmask = np.stack([dm_a, dm_b]).astype(np.float32)
        in_maps.append({
            "mg": mgb, "mg8": mg8s,
            "xg": np.ascontiguousarray(xTs[:, :, 0:512]).astype(NPBF16),
            "xg8": xg8s.astype(NPF8),
            "xkb": xkb_s.astype(NPBF16), "xk8": xkTs.astype(NPF8),
            "wvb": wv_b, "wv8": wv_b8,
            "dmask": np.ascontiguousarray(dmask).astype(NPBF16),
        })
    return in_maps


def _run(inputs, trace=False, **kw):
    global _NC
    if _NC is None:
        _NC = _build()
    x = np.asarray(inputs["x"], dtype=np.float32)
    Wq = np.asarray(inputs["Wq"], dtype=np.float32)
    Wk = np.asarray(inputs["Wk"], dtype=np.float32)
    Wv = np.asarray(inputs["Wv"], dtype=np.float32)
    in_maps = _prep_inputs(x, Wq, Wk, Wv)
    res = bass_utils.run_bass_kernel_spmd(
        _NC, in_maps, core_ids=list(range(8)), trace=trace, **kw)
    out = np.empty((B, S, D), dtype=np.float32)
    for b in range(B):
        po = (res.results[2 * b]["pout"].astype(np.float32)
              + res.results[2 * b + 1]["pout"].astype(np.float32))
        dn = res.results[2 * b]["den"] + res.results[2 * b + 1]["den"]
        out[b] = po / dn.T.reshape(S, 1)
    return out, res


def kernel(**inputs):
    out, _ = _run(inputs, trace=False)
    return out
